# revision 20
# baseline (speedup 1.0000x reference)
"""Causal self-attention (B=2048, T=128, C=192, H=6, D=32) on 8 TRN2 cores.

Data-parallel over batch: 256 elems/core. v2: zero DMA-xbar transposes —
all transposes ride the TensorE (transpose-mode matmuls), q^T/k^T are
computed directly as matmuls from x^T, and all three biases enter as
ones-rows in the contraction dimension. Only 2 HBM DMAs per element.

Per elem:
  x --cast--> x16 --PE transpose--> xT (+ones row)
  qT/kT = W^T @ xT (direct, bias fused);  v = x @ Wv (natural, bias fused)
  S_h[t,s] = -1e10*upper + q_h k_h^T   (mask via accumulating matmul)
  P = exp(S) on ScalarE (batched);  rowsums on VectorE; recip; normalize on
  GpSimd;  P^T via PE transpose;  y^T = V^T P^T (col-tiled);  out = y W_p
  (bias via ones-row) -> HBM.
"""

import sys

sys.path.insert(0, "/opt/trn_rl_repo")

import numpy as np
import ml_dtypes

N_CORES = 8
B, T, C = 2048, 128, 192
NH, HD = 6, 32
BL = B // N_CORES  # 256 per core

_CACHE = {}


def _build(bl):
    from contextlib import ExitStack

    import concourse.bass as bass
    import concourse.mybir as mybir
    import concourse.tile as tile
    from concourse import bacc

    fp32 = mybir.dt.float32
    bf16 = mybir.dt.bfloat16
    AF = mybir.ActivationFunctionType

    nc = bacc.Bacc("TRN2", target_bir_lowering=False, debug=False)

    x_d = nc.dram_tensor("x", [bl, T, C], fp32, kind="ExternalInput")
    wA_d = nc.dram_tensor("wA", [128, 704], bf16, kind="ExternalInput")
    wB_d = nc.dram_tensor("wB", [65, 704], bf16, kind="ExternalInput")
    wpA_d = nc.dram_tensor("wpA", [128, 192], bf16, kind="ExternalInput")
    wpB_d = nc.dram_tensor("wpB", [65, 192], bf16, kind="ExternalInput")
    mask_d = nc.dram_tensor("maskA", [128, 128], bf16, kind="ExternalInput")
    idr_d = nc.dram_tensor("identR", [128, 4, 128], bf16, kind="ExternalInput")
    out_d = nc.dram_tensor("out", [bl, T, C], fp32, kind="ExternalOutput")

    with tile.TileContext(nc) as tc, ExitStack() as ctx:
        consts = ctx.enter_context(tc.tile_pool(name="consts", bufs=1))
        sb = ctx.enter_context(tc.tile_pool(name="sb", bufs=6))
        ps = ctx.enter_context(
            tc.tile_pool(name="ps", bufs=1, space=bass.MemorySpace.PSUM)
        )

        wA = consts.tile([128, 704], bf16)
        nc.sync.dma_start(wA[:], wA_d[:])
        wB = consts.tile([65, 704], bf16)
        nc.sync.dma_start(wB[:], wB_d[:])
        wpA = consts.tile([128, 192], bf16)
        nc.sync.dma_start(wpA[:], wpA_d[:])
        wpB = consts.tile([65, 192], bf16)
        nc.sync.dma_start(wpB[:], wpB_d[:])
        maskA = consts.tile([128, 128], bf16)
        nc.sync.dma_start(maskA[:], mask_d[:])
        identR = consts.tile([128, 4, 128], bf16)
        nc.sync.dma_start(identR[:], idr_d[:])
        ident = identR[:, 0, :]

        def pt(tag, shape, dtype=fp32, name=None):
            return ps.tile(shape, dtype, tag=tag, name=name or f"ps_{tag}")

        for b in range(bl):
            xf = sb.tile([128, 192], fp32, tag="xf")
            nc.sync.dma_start(xf[:], x_d[b])
            x16 = sb.tile([128, 256], bf16, tag="x16")
            nc.vector.tensor_copy(x16[:, 0:192], xf[:])

            # x^T via PE transpose (2 chunks); col 192:256 junk is unread
            xTp = pt("xT", [128, 2, 128], bf16)
            nc.tensor.transpose(xTp[:, 0, :], x16[:, 0:128], ident)
            nc.tensor.transpose(xTp[:, 1, :], x16[:, 128:256], ident)
            xT = sb.tile([128, 2, 128], bf16, tag="xT")
            nc.vector.tensor_copy(xT[:], xTp[:])
            nc.gpsimd.memset(xT[64:65, 1, :], 1.0)  # ones row (bias)

            # q^T / k^T directly (cols host-permuted so every j is M=128
            # full-mode): j0=qT h0-3, j1=[qT h4-5; junk], j2=kT h0-3,
            # j3=[kT h4-5; junk]
            qkTp = pt("qkT", [128, 4, 128])
            for j in range(4):
                nc.tensor.matmul(
                    qkTp[:, j, :],
                    wA[:, 128 * j : 128 * (j + 1)],
                    xT[:, 0, :],
                    start=True,
                    stop=False,
                )
                nc.tensor.matmul(
                    qkTp[:, j, :],
                    wB[:, 128 * j : 128 * (j + 1)],
                    xT[0:65, 1, :],
                    start=False,
                    stop=True,
                )
            qkT = sb.tile([128, 4, 128], bf16, tag="qkT")
            nc.vector.tensor_copy(qkT[:], qkTp[:])

            # v natural [t, 192]
            vp = pt("v", [128, 192])
            nc.tensor.matmul(
                vp[:], xT[:, 0, :], wA[:, 512:704], start=True, stop=False
            )
            nc.tensor.matmul(
                vp[:], xT[0:65, 1, :], wB[:, 512:704], start=False, stop=True
            )
            v16 = sb.tile([128, 192], bf16, tag="v16")
            nc.scalar.copy(v16[:], vp[:])

            # S_h at [:, h, 0:128] of bank-strided PSUM (one bank per head:
            # concurrent row-tiles must not share a bank); mask first, then
            # row-tiled QK^T
            S4 = pt("xT", [128, 4, 512], name="S4")
            S2 = pt("qkT", [128, 2, 512], name="S2")
            Sv = [S4[:, h, 0:128] for h in range(4)] + [
                S2[:, h - 4, 0:128] for h in range(4, 6)
            ]
            for h in range(NH):
                nc.tensor.matmul(Sv[h], maskA[:], ident, start=True, stop=False)
            for h in range(NH):
                if h < 4:
                    q = qkT[h * 32 : h * 32 + 32, 0, :]
                    k = qkT[h * 32 : h * 32 + 32, 2, :]
                    r = h * 32
                else:
                    r = (h - 4) * 32
                    q = qkT[r : r + 32, 1, :]
                    k = qkT[r : r + 32, 3, :]
                nc.tensor.matmul(
                    Sv[h], q, k, start=False, stop=True, tile_position=(r, 0)
                )

            P16 = sb.tile([128, 6, 128], bf16, tag="P16")
            nc.scalar.activation(P16[:, 0:4, :], S4[:, :, 0:128], AF.Exp)
            nc.scalar.activation(P16[:, 4:6, :], S2[:, :, 0:128], AF.Exp)
            rsum = sb.tile([128, 8], fp32, tag="rsum")
            nc.vector.reduce_sum(
                rsum[:, 0:6], P16[:], axis=mybir.AxisListType.X
            )
            rrec = sb.tile([128, 8], fp32, tag="rrec")
            nc.vector.reciprocal(rrec[:, 0:6], rsum[:, 0:6])

            Pn = sb.tile([128, 6, 128], bf16, tag="Pn")
            for h in range(NH):
                nc.vector.tensor_scalar_mul(
                    Pn[:, h, :], P16[:, h, :], rrec[:, h : h + 1]
                )
            PTp = pt("v", [128, 6, 128], bf16, name="PTp")
            for h in range(NH):
                nc.tensor.transpose(PTp[:, h, :], Pn[:, h, :], ident)
            PT = sb.tile([128, 6, 128], bf16, tag="PT")
            nc.vector.tensor_copy(PT[:], PTp[:])

            # y^T: col-tiled, heads stacked on partitions
            yt = pt("S0", [128, 2, 128], name="yt")
            for h in range(NH):
                r = (h % 4) * 32
                j = 0 if h < 4 else 1
                nc.tensor.matmul(
                    yt[r : r + 32, j, :],
                    v16[:, h * 32 : h * 32 + 32],
                    PT[:, h, :],
                    start=True,
                    stop=True,
                    tile_position=(0, r),
                )
            yT = sb.tile([128, 2, 128], bf16, tag="yT")
            nc.vector.tensor_copy(yT[:], yt[:])
            nc.gpsimd.memset(yT[64:65, 1, :], 1.0)  # ones row (proj bias)

            outp = pt("S0", [128, 192], name="outp")
            nc.tensor.matmul(outp[:], yT[:, 0, :], wpA[:], start=True, stop=False)
            nc.tensor.matmul(
                outp[:], yT[0:65, 1, :], wpB[:], start=False, stop=True
            )
            outs = sb.tile([128, 192], fp32, tag="outs")
            nc.scalar.copy(outs[:], outp[:])
            nc.sync.dma_start(out_d[b], outs[:])

    nc.finalize()
    return nc


def _prep_inputs(x, w_qkv, b_qkv, w_proj, b_proj, bl):
    bf = ml_dtypes.bfloat16
    scale = 1.0 / np.sqrt(HD)
    w2 = np.array(w_qkv, dtype=np.float32, copy=True)
    b2 = np.array(b_qkv, dtype=np.float32, copy=True)
    w2[:, 0:C] *= scale
    b2[0:C] *= scale
    # column order: [q h0-3 | q h4-5 + pad | k h0-3 | k h4-5 + pad | v]
    # (pad cols produce junk in unread partitions, keeping M=128 full-mode)
    perm = np.concatenate(
        [
            np.arange(0, 128),
            np.arange(128, 192),
            np.arange(0, 64),
            np.arange(192, 320),
            np.arange(320, 384),
            np.arange(0, 64),
            np.arange(384, 576),
        ]
    )
    wA = w2[0:128][:, perm].astype(bf)
    wB = np.concatenate([w2[128:192], b2[None, :]], axis=0)[:, perm].astype(bf)
    wpA = np.asarray(w_proj)[0:128].astype(bf)
    wpB = np.concatenate(
        [np.asarray(w_proj)[128:192], np.asarray(b_proj)[None, :]], axis=0
    ).astype(bf)
    maskA = np.tril(np.full((128, 128), -1e10, np.float32), -1).astype(bf)
    identR = np.broadcast_to(
        np.eye(128, dtype=np.float32), (4, 128, 128)
    ).transpose(1, 0, 2)
    identR = np.ascontiguousarray(identR).astype(bf)
    xs = np.ascontiguousarray(np.asarray(x, dtype=np.float32)).reshape(
        -1, bl, T, C
    )
    maps = []
    for i in range(xs.shape[0]):
        maps.append(
            {
                "x": xs[i],
                "wA": wA,
                "wB": wB,
                "wpA": wpA,
                "wpB": wpB,
                "maskA": maskA,
                "identR": identR,
            }
        )
    return maps


def _run(x, w_qkv, b_qkv, w_proj, b_proj, bl=BL, n_cores=N_CORES, trace=False):
    from concourse.bass_utils import run_bass_kernel_spmd

    key = bl
    if key not in _CACHE:
        _CACHE[key] = _build(bl)
    nc = _CACHE[key]
    maps = _prep_inputs(x, w_qkv, b_qkv, w_proj, b_proj, bl)[:n_cores]
    res = run_bass_kernel_spmd(
        nc, maps, core_ids=list(range(len(maps))), trace=trace
    )
    out = np.concatenate([r["out"] for r in res.results], axis=0)
    return out, res


def kernel(x, w_qkv, b_qkv, w_proj, b_proj):
    out, _ = _run(x, w_qkv, b_qkv, w_proj, b_proj)
    return out.reshape(B, T, C).astype(np.float32)
